# revision 25
# baseline (speedup 1.0000x reference)
"""Trainium2 Bass kernel for nn_Decoder (GRU + concat-attention decoder).

Strategy: data-parallel over batch across 8 cores (8 examples/core).
Everything SBUF-resident; per-step recurrence fully unrolled.
Feature-on-partition layouts throughout; fp16 matmul operands, fp32
accumulation/elementwise. Readout/copy/maxout deferred to a batched
post-phase. Host does input layout prep (shard/transpose/cast) and
output reassembly; embedding gather runs on device via indirect DMA.
"""
import sys

sys.path.insert(0, "/opt/trn_rl_repo")


from contextlib import ExitStack

import numpy as np

import concourse.bacc as bacc
import concourse.bass as bass
import concourse.tile as tile
from concourse import mybir
from concourse.bass_utils import run_bass_kernel_spmd

F16 = mybir.dt.float16
F32 = mybir.dt.float32
I32 = mybir.dt.int32
AF = mybir.ActivationFunctionType
OP = mybir.AluOpType
AX = mybir.AxisListType

V, DW, ENC, DEC, ATT = 32000, 512, 512, 512, 512
T, B, L = 48, 64, 100
NC = 8
BL = B // NC  # 8 examples per core


def build_nc(t_steps=T):
    nc = bacc.Bacc("TRN2", target_bir_lowering=False)
    tb = t_steps * BL
    CH = min(128, tb)        # (t,b)-row chunk for gather/readout phases
    NCH = tb // CH
    TCH = CH // BL           # timesteps per row chunk

    # ---- DRAM inputs ----
    d_lut = nc.dram_tensor("word_lut", [V, DW], F32, kind="ExternalInput")
    d_ids = nc.dram_tensor("ids", [tb, 1], I32, kind="ExternalInput")
    d_wihe = nc.dram_tensor("wiheT", [DW, 3 * DEC], F16, kind="ExternalInput")
    d_wihc = nc.dram_tensor("wihcT", [ENC, 3 * DEC], F16, kind="ExternalInput")
    d_whh = nc.dram_tensor("whhT", [DEC, 3 * DEC], F16, kind="ExternalInput")
    d_wq = nc.dram_tensor("wqT", [DEC, ATT], F16, kind="ExternalInput")
    d_wpre = nc.dram_tensor("wpreT", [ENC, ATT], F16, kind="ExternalInput")
    d_wread = nc.dram_tensor("wreadT", [DW + DEC + ENC, DEC], F16, kind="ExternalInput")
    d_bread = nc.dram_tensor("bread", [1, DEC], F16, kind="ExternalInput")
    d_wcopy = nc.dram_tensor("wcopyT", [DEC + ENC, 1], F16, kind="ExternalInput")
    d_bcopy = nc.dram_tensor("bcopy", [1, 1], F16, kind="ExternalInput")
    d_gibias = nc.dram_tensor("gibiasT", [128, 12], F32, kind="ExternalInput")
    d_bhhn = nc.dram_tensor("bhhn", [1, DEC], F16, kind="ExternalInput")
    d_bpre = nc.dram_tensor("bpreT", [128, 4], F32, kind="ExternalInput")
    d_vshift = nc.dram_tensor("vshift", [128, 4 * BL * BL], F16, kind="ExternalInput")
    d_ctxst = nc.dram_tensor("ctxstT", [L, BL * ENC], F16, kind="ExternalInput")
    d_ctxmv = nc.dram_tensor("ctxmvT", [128, 4 * BL * L], F16, kind="ExternalInput")
    d_h0_32 = nc.dram_tensor("h0T32", [128, 4 * BL], F32, kind="ExternalInput")
    d_h0_16 = nc.dram_tensor("h0T16", [128, 4 * BL], F16, kind="ExternalInput")
    d_ia_16 = nc.dram_tensor("iaT16", [128, 4 * BL], F16, kind="ExternalInput")
    d_id128 = nc.dram_tensor("id128", [128, 128], F32, kind="ExternalInput")
    d_ones = nc.dram_tensor("ones1", [1, 128], F16, kind="ExternalInput")

    # ---- DRAM outputs ----
    o_mo = nc.dram_tensor("out_mo", [tb, 256], F32, kind="ExternalOutput")
    o_sc0 = nc.dram_tensor("out_score0", [BL // 2, t_steps * L], F32, kind="ExternalOutput")
    o_sc1 = nc.dram_tensor("out_score1", [BL // 2, t_steps * L], F32, kind="ExternalOutput")
    o_h = nc.dram_tensor("out_h", [128, t_steps * 4 * BL], F32, kind="ExternalOutput")
    o_wl = nc.dram_tensor("out_wctx_last", [128, 4 * BL], F32, kind="ExternalOutput")
    o_cp = nc.dram_tensor("out_cp", [tb, 1], F32, kind="ExternalOutput")

    with tile.TileContext(nc) as tc, ExitStack() as ctx:
        konst = ctx.enter_context(tc.tile_pool(name="konst", bufs=1))
        work = ctx.enter_context(tc.tile_pool(name="work", bufs=2))
        psL = ctx.enter_context(tc.tile_pool(name="psL", bufs=1, space="PSUM"))
        psP = psL

        def load(name, dram, shape, dtype, in_ap=None):
            t_ = konst.tile(shape, dtype, tag=name)
            nc.sync.dma_start(out=t_[:], in_=dram[:] if in_ap is None else in_ap)
            return t_

        wihe = load("wihe", d_wihe, [128, 4, 1536], F16,
                    d_wihe.rearrange("(k p) g -> p k g", p=128))
        wihc = load("wihc", d_wihc, [128, 4, 1536], F16,
                    d_wihc.rearrange("(k p) g -> p k g", p=128))
        whh = load("whh", d_whh, [128, 4, 1536], F16,
                   d_whh.rearrange("(k p) g -> p k g", p=128))
        wq = load("wq", d_wq, [128, 4, 512], F16,
                  d_wq.rearrange("(k p) g -> p k g", p=128))
        wpre = load("wpre", d_wpre, [128, 4, 512], F16,
                    d_wpre.rearrange("(k p) g -> p k g", p=128))
        wread = load("wread", d_wread, [128, 12, 512], F16,
                     d_wread.rearrange("(k p) g -> p k g", p=128))
        wcopy = load("wcopy", d_wcopy, [128, 8, 1], F16,
                     d_wcopy.rearrange("(k p) g -> p k g", p=128))
        bread = load("bread", d_bread, [1, 512], F16)
        bcopy = load("bcopy", d_bcopy, [1, 1], F16)
        gibias = load("gibias", d_gibias, [128, 12], F32)
        bhhn = load("bhhn", d_bhhn, [1, 4, 128], F16,
                    d_bhhn.rearrange("o (k p) -> o k p", p=128))
        bpre = load("bpre", d_bpre, [128, 4], F32)
        vshift = load("vshift", d_vshift, [128, 4, BL, BL], F16,
                      d_vshift.rearrange("p (k i j) -> p k i j", k=4, i=BL))
        ctxst = load("ctxst", d_ctxst, [L, BL, 4, 128], F16,
                     d_ctxst.rearrange("l (b k p) -> l b k p", b=BL, k=4))
        ctxmv = load("ctxmv", d_ctxmv, [128, 4, L, BL], F16,
                     d_ctxmv.rearrange("p (k l b) -> p k l b", k=4, l=L))
        h0_32 = load("h0_32", d_h0_32, [128, 4, BL], F32,
                     d_h0_32.rearrange("p (k b) -> p k b", k=4))
        h0_16 = load("h0_16", d_h0_16, [128, 4, BL], F16,
                     d_h0_16.rearrange("p (k b) -> p k b", k=4))
        ia16 = load("ia16", d_ia_16, [128, 4, BL], F16,
                    d_ia_16.rearrange("p (k b) -> p k b", k=4))
        id128 = load("id128", d_id128, [128, 128], F32)
        ones1 = load("ones1", d_ones, [1, 128], F16)
        ids_sb = load("ids_sb", d_ids, [CH, NCH, 1], I32,
                      d_ids.rearrange("(c p) o -> p c o", p=CH))

        # ---- persistent state / history tiles ----
        embT = konst.tile([128, 4, tb], F16, tag="embT")
        giemb = konst.tile([128, 12, t_steps, BL], F16, tag="giemb")
        preT = konst.tile([128, 4, L, BL], F16, tag="preT")
        hh32 = konst.tile([128, t_steps, 4, BL], F32, tag="hh32")
        hh16 = konst.tile([128, t_steps, 4, BL], F16, tag="hh16")
        wch16 = konst.tile([128, t_steps + 1, 4, BL], F16, tag="wch16")
        scg0 = konst.tile([BL // 2, t_steps, L], F32, tag="scg0")
        scg1 = konst.tile([BL // 2, t_steps, L], F32, tag="scg1")
        scg = [scg0, scg1]
        wlast = konst.tile([128, 4, BL], F32, tag="wlast")
        embrows = konst.tile([CH, NCH, 512], F32, tag="embrows")

        # ---------------- Phase A: embedding gather + transpose ----------------
        for c in range(NCH):
            nc.gpsimd.indirect_dma_start(
                out=embrows[:, c, :],
                out_offset=None,
                in_=d_lut[:],
                in_offset=bass.IndirectOffsetOnAxis(ap=ids_sb[:, c, :], axis=0),
            )
        for c in range(NCH):
            for kc in range(4):
                pt = psL.tile([128, CH], F32, tag="small1")
                nc.tensor.transpose(pt[:], embrows[:, c, kc * 128:(kc + 1) * 128],
                                    id128[:CH, :CH])
                nc.vector.tensor_copy(embT[:, kc, c * CH:(c + 1) * CH], pt[:])

        # ---------------- Phase B: gi_emb = W_ihe @ emb + biases ----------------
        for h4 in range(6):
            pg = psP.tile([128, 2, 512], F32, tag="big")
            for g3 in range(2):
                gc = 2 * h4 + g3
                for kc in range(4):
                    nc.tensor.matmul(
                        pg[:, g3, :tb], lhsT=wihe[:, kc, gc * 128:(gc + 1) * 128],
                        rhs=embT[:, kc, :], start=(kc == 0), stop=(kc == 3))
            nc.vector.tensor_tensor(
                out=giemb[:, 2 * h4:2 * h4 + 2, :, :].rearrange("p a b c -> p a (b c)"),
                in0=pg[:, :, :tb],
                in1=gibias[:, 2 * h4:2 * h4 + 2].to_broadcast([128, 2, tb]),
                op=OP.add)

        # ---------------- Phase C: pre = W_pre @ ctx + b_pre ----------------
        for ac in range(4):
            for lh in range(2):
                pp_t = psP.tile([128, 2, 512], F32, tag="big", name="pp_t")
                pp = pp_t[:, 0, :]
                for kc in range(4):
                    nc.tensor.matmul(
                        pp[:, :50 * BL],
                        lhsT=wpre[:, kc, ac * 128:(ac + 1) * 128],
                        rhs=ctxmv[:, kc, 50 * lh:50 * lh + 50, :].rearrange("p l b -> p (l b)"),
                        start=(kc == 0), stop=(kc == 3))
                nc.scalar.activation(
                    out=preT[:, ac, 50 * lh:50 * lh + 50, :].rearrange("p l b -> p (l b)"),
                    in_=pp[:, :50 * BL], func=AF.Identity, bias=bpre[:, ac:ac + 1])

        # ---------------- Phase D: recurrence ----------------
        # Two independent batch sub-groups of 4 pipeline against each other:
        # group A's DVE/ACT attention overlaps group B's PE gate matmuls.
        GB = BL // 2
        for t in range(t_steps):
          for g in range(2):
            bs = slice(GB * g, GB * (g + 1))
            hp16 = (lambda kc: h0_16[:, kc, bs]) if t == 0 else \
                (lambda kc, _t=t: hh16[:, kc, _t - 1, bs])
            hprev32 = h0_32[:, :, bs] if t == 0 else hh32[:, t - 1, :, bs]
            wp16 = (lambda kc: ia16[:, kc, bs]) if t == 0 else \
                (lambda kc, _t=t: wch16[:, kc, _t, bs])

            # one PSUM bank per group: [gates 16 | q 4 | wctx 4] x GB
            psZ = psL.tile([128, 24, GB], F32, tag=f"psZ{g}")
            psG = psZ[:, 0:16, :]
            psA = psZ[:, 0:8, :]
            psB = psZ[:, 8:12, :]
            psC = psZ[:, 12:16, :]
            psQ = psZ[:, 16:20, :]
            psW = psZ[:, 20:24, :]
            psET = psL.tile([128, 128, GB], F32, tag=f"psET{g}")
            psE = psET[0:GB, 0:25, :].rearrange("p a b -> p (a b)")

            for gc in range(8):
                for kc in range(4):
                    nc.tensor.matmul(psA[:, gc, :], lhsT=whh[:, kc, gc * 128:(gc + 1) * 128],
                                     rhs=hp16(kc), start=(kc == 0), stop=False)
                for kc in range(4):
                    nc.tensor.matmul(psA[:, gc, :], lhsT=wihc[:, kc, gc * 128:(gc + 1) * 128],
                                     rhs=wp16(kc), start=False, stop=(kc == 3))
            for gn in range(4):
                gc = 8 + gn
                for kc in range(4):
                    nc.tensor.matmul(psB[:, gn, :], lhsT=wihc[:, kc, gc * 128:(gc + 1) * 128],
                                     rhs=wp16(kc), start=(kc == 0), stop=(kc == 3))
                for kc in range(4):
                    nc.tensor.matmul(psC[:, gn, :], lhsT=whh[:, kc, gc * 128:(gc + 1) * 128],
                                     rhs=hp16(kc), start=(kc == 0), stop=False)
                nc.tensor.matmul(psC[:, gn, :], lhsT=bhhn[:, gn, :], rhs=ones1[:, :GB],
                                 start=False, stop=True)

            # --- GRU elementwise ---
            rzarg = work.tile([128, 8, GB], F32, tag=f"rzarg{g}")
            nc.vector.tensor_tensor(out=rzarg[:], in0=psA[:], in1=giemb[:, 0:8, t, bs],
                                    op=OP.add)
            rz = work.tile([128, 8, GB], F32, tag=f"rz{g}")
            nc.scalar.activation(out=rz[:], in_=rzarg[:], func=AF.Tanh, scale=0.5)
            nc.vector.tensor_scalar(out=rz[:], in0=rz[:], scalar1=0.5, scalar2=0.5,
                                    op0=OP.mult, op1=OP.add)
            t1 = work.tile([128, 4, GB], F32, tag=f"t1{g}")
            nc.vector.tensor_tensor(out=t1[:], in0=psC[:], in1=rz[:, 0:4, :], op=OP.mult)
            nc.vector.tensor_tensor(out=t1[:], in0=t1[:], in1=giemb[:, 8:12, t, bs], op=OP.add)
            narg = work.tile([128, 4, GB], F32, tag=f"narg{g}")
            nc.vector.tensor_tensor(out=narg[:], in0=t1[:], in1=psB[:], op=OP.add)
            nn_ = work.tile([128, 4, GB], F32, tag=f"nn{g}")
            nc.scalar.activation(out=nn_[:], in_=narg[:], func=AF.Tanh)
            dd = work.tile([128, 4, GB], F32, tag=f"dd{g}")
            nc.vector.tensor_tensor(out=dd[:], in0=hprev32, in1=nn_[:], op=OP.subtract)
            nc.vector.tensor_tensor(out=dd[:], in0=dd[:], in1=rz[:, 4:8, :], op=OP.mult)
            nc.vector.tensor_tensor(out=hh32[:, t, :, bs], in0=nn_[:], in1=dd[:], op=OP.add)
            nc.vector.tensor_copy(hh16[:, :, t, bs], hh32[:, t, :, bs])

            # --- q = h1 @ W_q.T ---
            for ac in range(4):
                for kc in range(4):
                    nc.tensor.matmul(psQ[:, ac, :], lhsT=wq[:, kc, ac * 128:(ac + 1) * 128],
                                     rhs=hh16[:, kc, t, bs], start=(kc == 0), stop=(kc == 3))

            # --- arg = pre + q (broadcast over l), tanh, energy ---
            arg16 = work.tile([128, 4, L, GB], F16, tag=f"arg16{g}")
            s16 = work.tile([128, 4, L, GB], F16, tag=f"s16{g}")
            q16 = work.tile([128, 4, 1, GB], F16, tag=f"q16{g}")
            nc.vector.tensor_copy(q16[:, :, 0, :], psQ[:])
            i_mm = 0
            for ac in range(4):
                nc.vector.tensor_tensor(
                    out=arg16[:, ac, :, :], in0=preT[:, ac, :, bs],
                    in1=q16[:, ac, :, :].to_broadcast([128, L, GB]),
                    op=OP.add)
                nc.scalar.activation(
                    out=s16[:, ac, :, :], in_=arg16[:, ac, :, :], func=AF.Tanh)
                for b in range(GB):
                    nc.tensor.matmul(psE[:], lhsT=vshift[:, ac, GB * g + b, bs],
                                     rhs=s16[:, ac, :, b],
                                     start=(i_mm == 0), stop=(i_mm == 4 * GB - 1))
                    i_mm += 1

            # --- softmax over l ---
            usum = work.tile([GB, 1], F32, tag=f"usum{g}")
            nc.scalar.activation(out=scg[g][:, t, :], in_=psE[:], func=AF.Exp,
                                 accum_out=usum[:])
            rcp = work.tile([GB, 1], F32, tag=f"rcp{g}")
            nc.vector.reciprocal(rcp[:], usum[:])
            nc.vector.tensor_scalar_mul(scg[g][:, t, :], scg[g][:, t, :], rcp[:])

            # --- score transpose -> [l, b], wctx ---
            psT = psET[0:L, 25:26, :].rearrange("p a b -> p (a b)")
            nc.tensor.transpose(psT, scg[g][:, t, :], id128[:GB, :GB])
            scT = work.tile([L, GB], F16, tag=f"scT{g}")
            nc.vector.tensor_copy(scT[:], psT)
            for ec in range(4):
                for b in range(GB):
                    nc.tensor.matmul(psW[:, ec, b:b + 1], lhsT=ctxst[:, GB * g + b, ec, :],
                                     rhs=scT[:, b:b + 1], start=True, stop=True)
            nc.vector.tensor_copy(wch16[:, :, t + 1, bs], psW[:])
            if t == t_steps - 1:
                nc.vector.tensor_copy(wlast[:, :, bs], psW[:])

        # -------- Phase E: deferred readout / copy gate / maxout --------
        for c in range(NCH):
            t0 = c * TCH
            pR_t = psP.tile([128, 2, 512], F32, tag="big", name="pR_t")
            pR = pR_t[:, 0, :]
            pC = psP.tile([128, 1], F32, tag="small2")

            def xcat(kc):
                if kc < 4:
                    return embT[:, kc, c * CH:(c + 1) * CH]
                if kc < 8:
                    return hh16[:, t0:t0 + TCH, kc - 4, :]
                return wch16[:, t0 + 1:t0 + TCH + 1, kc - 8, :]

            for kc in range(12):
                nc.tensor.matmul(pR[:CH, :], lhsT=xcat(kc), rhs=wread[:, kc, :],
                                 start=(kc == 0), stop=False)
            nc.tensor.matmul(pR[:CH, :], lhsT=ones1[:, :CH], rhs=bread[:],
                             start=False, stop=True)
            for kc in range(4, 12):
                nc.tensor.matmul(pC[:CH, :], lhsT=xcat(kc), rhs=wcopy[:, kc - 4, :],
                                 start=(kc == 4), stop=False)
            nc.tensor.matmul(pC[:CH, :], lhsT=ones1[:, :CH], rhs=bcopy[:],
                             start=False, stop=True)
            roA = work.tile([CH, 256], F32, tag="roA")
            nc.scalar.copy(roA[:], pR[:CH, 0:256])
            mo = work.tile([CH, 256], F32, tag="mo")
            nc.vector.tensor_tensor(out=mo[:], in0=roA[:], in1=pR[:CH, 256:512], op=OP.max)
            nc.sync.dma_start(out=o_mo[c * CH:(c + 1) * CH, :], in_=mo[:])
            cpt = work.tile([CH, 1], F32, tag="cpt")
            nc.scalar.activation(out=cpt[:], in_=pC[:CH, :], func=AF.Tanh, scale=0.5)
            nc.vector.tensor_scalar(out=cpt[:], in0=cpt[:], scalar1=0.5, scalar2=0.5,
                                    op0=OP.mult, op1=OP.add)
            nc.sync.dma_start(out=o_cp[c * CH:(c + 1) * CH, :], in_=cpt[:])

        # ---------------- final DMAs ----------------
        nc.sync.dma_start(out=o_h[:], in_=hh32[:].rearrange("p t k b -> p (t k b)"))
        nc.sync.dma_start(out=o_sc0[:], in_=scg0[:].rearrange("b t l -> b (t l)"))
        nc.sync.dma_start(out=o_sc1[:], in_=scg1[:].rearrange("b t l -> b (t l)"))
        nc.sync.dma_start(out=o_wl[:], in_=wlast[:].rearrange("p k b -> p (k b)"))

    nc.compile()
    return nc


# ============================ host side ============================

def _prep_core_inputs(inputs, core, t_steps=T):
    b0 = core * BL
    f32 = lambda k: np.asarray(inputs[k], np.float32)
    w_ih, w_hh = f32("W_ih"), f32("W_hh")
    b_ih, b_hh = f32("b_ih"), f32("b_hh")
    w_read, b_read = f32("W_read"), f32("b_read")
    w_copy, b_copy = f32("W_copy"), f32("b_copy")
    w_pre, b_pre = f32("W_pre"), f32("b_pre")
    w_q, w_v = f32("W_q"), f32("W_v")
    ctx = f32("context")[:, b0:b0 + BL, :]                      # [L, BL, E]
    ids = np.asarray(inputs["input_ids"]).astype(np.int32)[:t_steps, b0:b0 + BL]
    h0 = f32("hidden")[0, b0:b0 + BL, :]                        # [BL, D]
    ia = f32("init_att")[b0:b0 + BL, :]

    perm = np.concatenate([np.arange(0, DEC, 2), np.arange(1, DEC, 2)])
    w_read_r, b_read_r = w_read[perm], b_read[perm]

    gibias = np.concatenate([(b_ih + b_hh)[:2 * DEC], b_ih[2 * DEC:]])
    vshift = np.zeros((4, 128, BL, BL), np.float16)
    for c in range(4):
        for b in range(BL):
            vshift[c, :, b, b] = w_v[0, c * 128:(c + 1) * 128].astype(np.float16)

    def chunkT(x):  # [N, D=512] -> [128, 4, N] -> [128, 4*N]
        return np.ascontiguousarray(x.T).reshape(4, 128, -1).transpose(1, 0, 2) \
                 .reshape(128, -1)

    d = {
        "word_lut": f32("word_lut"),
        "ids": ids.reshape(-1, 1),
        "wiheT": np.ascontiguousarray(w_ih[:, :DW].T).astype(np.float16),
        "wihcT": np.ascontiguousarray(w_ih[:, DW:].T).astype(np.float16),
        "whhT": np.ascontiguousarray(w_hh.T).astype(np.float16),
        "wqT": np.ascontiguousarray(w_q.T).astype(np.float16),
        "wpreT": np.ascontiguousarray(w_pre.T).astype(np.float16),
        "wreadT": np.ascontiguousarray(w_read_r.T).astype(np.float16),
        "bread": b_read_r.reshape(1, -1).astype(np.float16),
        "wcopyT": np.ascontiguousarray(w_copy.T).astype(np.float16),
        "bcopy": b_copy.reshape(1, 1).astype(np.float16),
        "gibiasT": np.ascontiguousarray(gibias.reshape(12, 128).T).astype(np.float32),
        "bhhn": b_hh[2 * DEC:].reshape(1, -1).astype(np.float16),
        "bpreT": np.ascontiguousarray(b_pre.reshape(4, 128).T).astype(np.float32),
        "vshift": vshift.transpose(1, 0, 2, 3).reshape(128, -1),
        "ctxstT": ctx.reshape(L, -1).astype(np.float16),
        "ctxmvT": np.ascontiguousarray(ctx.transpose(2, 0, 1)).reshape(4, 128, L, BL)
                    .transpose(1, 0, 2, 3).reshape(128, -1).astype(np.float16),
        "h0T32": chunkT(h0).astype(np.float32),
        "id128": np.eye(128, dtype=np.float32),
        "ones1": np.ones((1, 128), np.float16),
    }
    d["h0T16"] = d["h0T32"].astype(np.float16)
    d["iaT16"] = chunkT(ia).astype(np.float16)
    return d


def _assemble(results, t_steps=T):
    g_outputs = np.zeros((t_steps, B, 256), np.float32)
    c_outputs = np.zeros((t_steps, B, L), np.float32)
    copy_gates = np.zeros((t_steps, B, 1), np.float32)
    g_hiddens = np.zeros((t_steps, 1, B, DEC), np.float32)
    ctx_fin = np.zeros((B, ENC), np.float32)
    for c in range(NC):
        r = results[c]
        b0 = c * BL
        g_outputs[:, b0:b0 + BL, :] = r["out_mo"].reshape(t_steps, BL, 256)
        sc = np.concatenate([r["out_score0"].reshape(BL // 2, t_steps, L),
                             r["out_score1"].reshape(BL // 2, t_steps, L)], axis=0)
        c_outputs[:, b0:b0 + BL, :] = sc.transpose(1, 0, 2)
        copy_gates[:, b0:b0 + BL, :] = r["out_cp"].reshape(t_steps, BL, 1)
        hh = r["out_h"].reshape(128, t_steps, 4, BL)
        g_hiddens[:, 0, b0:b0 + BL, :] = hh.transpose(1, 3, 2, 0).reshape(t_steps, BL, DEC)
        wl = r["out_wctx_last"].reshape(128, 4, BL)
        ctx_fin[b0:b0 + BL, :] = wl.transpose(2, 1, 0).reshape(BL, ENC)
    h_fin = g_hiddens[t_steps - 1]
    attn_last = c_outputs[t_steps - 1]
    return (g_outputs, c_outputs, copy_gates, h_fin, attn_last, ctx_fin, g_hiddens)


_NC_CACHE = {}


def kernel(**inputs):
    t_steps = T
    if t_steps not in _NC_CACHE:
        _NC_CACHE[t_steps] = build_nc(t_steps)
    nc = _NC_CACHE[t_steps]
    in_maps = [_prep_core_inputs(inputs, c, t_steps) for c in range(NC)]
    res = run_bass_kernel_spmd(nc, in_maps, core_ids=list(range(NC)))
    return _assemble(res.results, t_steps)


# revision 32
# speedup vs baseline: 1.0726x; 1.0726x over previous
"""Trainium2 Bass kernel for nn_Decoder (GRU + concat-attention decoder).

Strategy: data-parallel over batch across 8 cores (8 examples/core).
Everything SBUF-resident; per-step recurrence fully unrolled.
Feature-on-partition layouts throughout; fp16 matmul operands, fp32
accumulation/elementwise. Readout/copy/maxout deferred to a batched
post-phase. Host does input layout prep (shard/transpose/cast) and
output reassembly; embedding gather runs on device via indirect DMA.
"""
import sys

sys.path.insert(0, "/opt/trn_rl_repo")


from contextlib import ExitStack

import numpy as np

import concourse.bacc as bacc
import concourse.bass as bass
import concourse.tile as tile
from concourse import mybir
from concourse.bass_utils import run_bass_kernel_spmd

F16 = mybir.dt.float16
F32 = mybir.dt.float32
I32 = mybir.dt.int32
AF = mybir.ActivationFunctionType
OP = mybir.AluOpType
AX = mybir.AxisListType

V, DW, ENC, DEC, ATT = 32000, 512, 512, 512, 512
T, B, L = 48, 64, 100
NC = 8
BL = B // NC  # 8 examples per core


def build_nc(t_steps=T):
    nc = bacc.Bacc("TRN2", target_bir_lowering=False)
    tb = t_steps * BL
    CH = min(128, tb)        # (t,b)-row chunk for gather/readout phases
    NCH = tb // CH
    TCH = CH // BL           # timesteps per row chunk

    # ---- DRAM inputs ----
    d_lut = nc.dram_tensor("word_lut", [V, DW], F32, kind="ExternalInput")
    d_ids = nc.dram_tensor("ids", [tb, 1], I32, kind="ExternalInput")
    d_wihe = nc.dram_tensor("wiheT", [DW, 3 * DEC], F16, kind="ExternalInput")
    d_wihc = nc.dram_tensor("wihcT", [ENC, 3 * DEC], F16, kind="ExternalInput")
    d_whh = nc.dram_tensor("whhT", [DEC, 3 * DEC], F16, kind="ExternalInput")
    d_wq = nc.dram_tensor("wqT", [DEC, ATT], F16, kind="ExternalInput")
    d_wpre = nc.dram_tensor("wpreT", [ENC, ATT], F16, kind="ExternalInput")
    d_wread = nc.dram_tensor("wreadT", [DW + DEC + ENC, DEC], F16, kind="ExternalInput")
    d_bread = nc.dram_tensor("bread", [1, DEC], F16, kind="ExternalInput")
    d_wcopy = nc.dram_tensor("wcopyT", [DEC + ENC, 1], F16, kind="ExternalInput")
    d_bcopy = nc.dram_tensor("bcopy", [1, 1], F16, kind="ExternalInput")
    d_gibias = nc.dram_tensor("gibiasT", [128, 12], F32, kind="ExternalInput")
    d_bhhn = nc.dram_tensor("bhhn", [1, DEC], F16, kind="ExternalInput")
    d_bpre = nc.dram_tensor("bpreT", [128, 4], F32, kind="ExternalInput")
    d_vshift = nc.dram_tensor("vshift", [128, 4 * BL * BL], F16, kind="ExternalInput")
    d_ctxst = nc.dram_tensor("ctxstT", [L, BL * ENC], F16, kind="ExternalInput")
    d_ctxmv = nc.dram_tensor("ctxmvT", [128, 4 * BL * L], F16, kind="ExternalInput")
    d_h0_32 = nc.dram_tensor("h0T32", [128, 4 * BL], F32, kind="ExternalInput")
    d_h0_16 = nc.dram_tensor("h0T16", [128, 4 * BL], F16, kind="ExternalInput")
    d_ia_16 = nc.dram_tensor("iaT16", [128, 4 * BL], F16, kind="ExternalInput")
    d_id128 = nc.dram_tensor("id128", [128, 128], F32, kind="ExternalInput")
    d_id128h = nc.dram_tensor("id128h", [128, 128], F16, kind="ExternalInput")
    d_ones = nc.dram_tensor("ones1", [1, 128], F16, kind="ExternalInput")

    # ---- DRAM outputs ----
    o_mo = nc.dram_tensor("out_mo", [tb, 256], F32, kind="ExternalOutput")
    o_sc0 = nc.dram_tensor("out_score0", [BL // 2, t_steps * L], F32, kind="ExternalOutput")
    o_sc1 = nc.dram_tensor("out_score1", [BL // 2, t_steps * L], F32, kind="ExternalOutput")
    o_h = nc.dram_tensor("out_h", [128, t_steps * 4 * BL], F32, kind="ExternalOutput")
    o_wl = nc.dram_tensor("out_wctx_last", [128, 4 * BL], F32, kind="ExternalOutput")
    o_cp = nc.dram_tensor("out_cp", [tb, 1], F32, kind="ExternalOutput")

    with tile.TileContext(nc) as tc, ExitStack() as ctx:
        konst = ctx.enter_context(tc.tile_pool(name="konst", bufs=1))
        work = ctx.enter_context(tc.tile_pool(name="work", bufs=2))
        psL = ctx.enter_context(tc.tile_pool(name="psL", bufs=1, space="PSUM"))
        psP = psL

        def load(name, dram, shape, dtype, in_ap=None):
            t_ = konst.tile(shape, dtype, tag=name)
            nc.sync.dma_start(out=t_[:], in_=dram[:] if in_ap is None else in_ap)
            return t_

        wihe = load("wihe", d_wihe, [128, 4, 1536], F16,
                    d_wihe.rearrange("(k p) g -> p k g", p=128))
        wihc = load("wihc", d_wihc, [128, 4, 1536], F16,
                    d_wihc.rearrange("(k p) g -> p k g", p=128))
        whh = load("whh", d_whh, [128, 4, 1536], F16,
                   d_whh.rearrange("(k p) g -> p k g", p=128))
        wq = load("wq", d_wq, [128, 4, 512], F16,
                  d_wq.rearrange("(k p) g -> p k g", p=128))
        wpre = load("wpre", d_wpre, [128, 4, 512], F16,
                    d_wpre.rearrange("(k p) g -> p k g", p=128))
        wread = load("wread", d_wread, [128, 12, 512], F16,
                     d_wread.rearrange("(k p) g -> p k g", p=128))
        wcopy = load("wcopy", d_wcopy, [128, 8, 1], F16,
                     d_wcopy.rearrange("(k p) g -> p k g", p=128))
        bread = load("bread", d_bread, [1, 512], F16)
        bcopy = load("bcopy", d_bcopy, [1, 1], F16)
        gibias = load("gibias", d_gibias, [128, 12], F32)
        bhhn = load("bhhn", d_bhhn, [1, 4, 128], F16,
                    d_bhhn.rearrange("o (k p) -> o k p", p=128))
        bpre = load("bpre", d_bpre, [128, 4], F32)
        vshift = load("vshift", d_vshift, [128, 4, BL, BL], F16,
                      d_vshift.rearrange("p (k i j) -> p k i j", k=4, i=BL))
        ctxst = load("ctxst", d_ctxst, [L, BL, 4, 128], F16,
                     d_ctxst.rearrange("l (b k p) -> l b k p", b=BL, k=4))
        ctxmv = load("ctxmv", d_ctxmv, [128, 4, L, BL], F16,
                     d_ctxmv.rearrange("p (k l b) -> p k l b", k=4, l=L))
        h0_32 = load("h0_32", d_h0_32, [128, 4, BL], F32,
                     d_h0_32.rearrange("p (k b) -> p k b", k=4))
        h0_16 = load("h0_16", d_h0_16, [128, 4, BL], F16,
                     d_h0_16.rearrange("p (k b) -> p k b", k=4))
        ia16 = load("ia16", d_ia_16, [128, 4, BL], F16,
                    d_ia_16.rearrange("p (k b) -> p k b", k=4))
        id128 = load("id128", d_id128, [128, 128], F32)
        id128h = load("id128h", d_id128h, [128, 128], F16)
        ones1 = load("ones1", d_ones, [1, 128], F16)
        ids_sb = load("ids_sb", d_ids, [CH, NCH, 1], I32,
                      d_ids.rearrange("(c p) o -> p c o", p=CH))

        # ---- persistent state / history tiles ----
        embT = konst.tile([128, 4, tb], F16, tag="embT")
        giemb = konst.tile([128, 12, t_steps, BL], F16, tag="giemb")
        preT = konst.tile([128, 4, L, BL], F16, tag="preT")
        hcast = konst.tile([128, 4, t_steps, BL], F32, tag="hcast")
        hh16 = konst.tile([128, t_steps, 4, BL], F16, tag="hh16")
        wch16 = konst.tile([128, t_steps + 1, 4, BL], F16, tag="wch16")
        scg0 = konst.tile([BL // 2, t_steps, L], F32, tag="scg0")
        scg1 = konst.tile([BL // 2, t_steps, L], F32, tag="scg1")
        scg = [scg0, scg1]
        wlast = konst.tile([128, 4, BL], F32, tag="wlast")
        embrows = konst.tile([CH, NCH, 512], F32, tag="embrows")

        # ---------------- Phase A: embedding gather + transpose ----------------
        for c in range(NCH):
            nc.gpsimd.indirect_dma_start(
                out=embrows[:, c, :],
                out_offset=None,
                in_=d_lut[:],
                in_offset=bass.IndirectOffsetOnAxis(ap=ids_sb[:, c, :], axis=0),
            )
        for c in range(NCH):
            for kc in range(4):
                pt = psL.tile([128, CH], F32, tag="small1")
                nc.tensor.transpose(pt[:], embrows[:, c, kc * 128:(kc + 1) * 128],
                                    id128[:CH, :CH])
                nc.vector.tensor_copy(embT[:, kc, c * CH:(c + 1) * CH], pt[:])

        # ---------------- Phase B: gi_emb = W_ihe @ emb + biases ----------------
        for h4 in range(6):
            pg = psP.tile([128, 2, 512], F32, tag="big")
            for g3 in range(2):
                gc = 2 * h4 + g3
                for kc in range(4):
                    nc.tensor.matmul(
                        pg[:, g3, :tb], lhsT=wihe[:, kc, gc * 128:(gc + 1) * 128],
                        rhs=embT[:, kc, :], start=(kc == 0), stop=(kc == 3))
            nc.vector.tensor_tensor(
                out=giemb[:, 2 * h4:2 * h4 + 2, :, :].rearrange("p a b c -> p a (b c)"),
                in0=pg[:, :, :tb],
                in1=gibias[:, 2 * h4:2 * h4 + 2].to_broadcast([128, 2, tb]),
                op=OP.add)

        # ---------------- Phase C: pre = W_pre @ ctx + b_pre ----------------
        for ac in range(4):
            for lh in range(2):
                pp_t = psP.tile([128, 2, 512], F32, tag="big", name="pp_t")
                pp = pp_t[:, 0, :]
                for kc in range(4):
                    nc.tensor.matmul(
                        pp[:, :50 * BL],
                        lhsT=wpre[:, kc, ac * 128:(ac + 1) * 128],
                        rhs=ctxmv[:, kc, 50 * lh:50 * lh + 50, :].rearrange("p l b -> p (l b)"),
                        start=(kc == 0), stop=(kc == 3))
                nc.scalar.activation(
                    out=preT[:, ac, 50 * lh:50 * lh + 50, :].rearrange("p l b -> p (l b)"),
                    in_=pp[:, :50 * BL], func=AF.Identity, bias=bpre[:, ac:ac + 1])

        # ---------------- Phase D: recurrence ----------------
        # Two independent batch sub-groups of 4 pipeline against each other:
        # group A's DVE/ACT attention overlaps group B's PE gate matmuls.
        GB = BL // 2
        for t in range(t_steps):
          for g in range(2):
            bs = slice(GB * g, GB * (g + 1))
            hp16 = (lambda kc: h0_16[:, kc, bs]) if t == 0 else \
                (lambda kc, _t=t: hh16[:, kc, _t - 1, bs])
            hprev16 = h0_16[:, :, bs] if t == 0 else hh16[:, :, t - 1, bs]
            wp16 = (lambda kc: ia16[:, kc, bs]) if t == 0 else \
                (lambda kc, _t=t: wch16[:, kc, _t, bs])

            # one PSUM bank per group: [gates 16 | q 4 | wctx 4] x GB
            psZ = psL.tile([128, 24, GB], F32, tag=f"psZ{g}")
            psG = psZ[:, 0:16, :]
            psA = psZ[:, 0:8, :]
            psB = psZ[:, 8:12, :]
            psC = psZ[:, 12:16, :]
            psQ = psZ[:, 16:20, :]
            psW = psZ[:, 20:24, :]
            psET = psL.tile([128, 128, GB], F32, tag=f"psET{g}")
            psE = psET[0:GB, 0:25, :].rearrange("p a b -> p (a b)")

            for gc in range(8):
                for kc in range(4):
                    nc.tensor.matmul(psA[:, gc, :], lhsT=whh[:, kc, gc * 128:(gc + 1) * 128],
                                     rhs=hp16(kc), start=(kc == 0), stop=False)
                for kc in range(4):
                    nc.tensor.matmul(psA[:, gc, :], lhsT=wihc[:, kc, gc * 128:(gc + 1) * 128],
                                     rhs=wp16(kc), start=False, stop=False)
                nc.tensor.matmul(psA[:, gc, :], lhsT=id128h[:], rhs=giemb[:, gc, t, bs],
                                 start=False, stop=True)
            for gn in range(4):
                gc = 8 + gn
                for kc in range(4):
                    nc.tensor.matmul(psB[:, gn, :], lhsT=wihc[:, kc, gc * 128:(gc + 1) * 128],
                                     rhs=wp16(kc), start=(kc == 0), stop=False)
                nc.tensor.matmul(psB[:, gn, :], lhsT=id128h[:], rhs=giemb[:, 8 + gn, t, bs],
                                 start=False, stop=True)
                for kc in range(4):
                    nc.tensor.matmul(psC[:, gn, :], lhsT=whh[:, kc, gc * 128:(gc + 1) * 128],
                                     rhs=hp16(kc), start=(kc == 0), stop=False)
                nc.tensor.matmul(psC[:, gn, :], lhsT=bhhn[:, gn, :], rhs=ones1[:, :GB],
                                 start=False, stop=True)

            # --- GRU elementwise ---
            rz = work.tile([128, 8, GB], F32, tag=f"rz{g}")
            nc.scalar.activation(out=rz[:], in_=psA[:], func=AF.Tanh, scale=0.5)
            nc.vector.tensor_scalar(out=rz[:], in0=rz[:], scalar1=0.5, scalar2=0.5,
                                    op0=OP.mult, op1=OP.add)
            t1 = work.tile([128, 4, GB], F32, tag=f"t1{g}")
            nc.vector.tensor_tensor(out=t1[:], in0=psC[:], in1=rz[:, 0:4, :], op=OP.mult)
            narg = work.tile([128, 4, GB], F32, tag=f"narg{g}")
            nc.vector.tensor_tensor(out=narg[:], in0=t1[:], in1=psB[:], op=OP.add)
            nn_ = work.tile([128, 4, GB], F32, tag=f"nn{g}")
            nc.scalar.activation(out=nn_[:], in_=narg[:], func=AF.Tanh)
            dd = work.tile([128, 4, GB], F32, tag=f"dd{g}")
            nc.vector.tensor_tensor(out=dd[:], in0=hprev16, in1=nn_[:], op=OP.subtract)
            nc.vector.tensor_tensor(out=dd[:], in0=dd[:], in1=rz[:, 4:8, :], op=OP.mult)
            nc.vector.tensor_tensor(out=hh16[:, :, t, bs], in0=nn_[:], in1=dd[:], op=OP.add)

            # --- q = h1 @ W_q.T ---
            for ac in range(4):
                for kc in range(4):
                    nc.tensor.matmul(psQ[:, ac, :], lhsT=wq[:, kc, ac * 128:(ac + 1) * 128],
                                     rhs=hh16[:, kc, t, bs], start=(kc == 0), stop=(kc == 3))

            # --- arg = pre + q (broadcast over l), tanh, energy ---
            arg16 = work.tile([128, 4, L, GB], F16, tag=f"arg16{g}")
            s16 = work.tile([128, 4, L, GB], F16, tag=f"s16{g}")
            q16 = work.tile([128, 4, 1, GB], F16, tag=f"q16{g}")
            nc.vector.tensor_copy(q16[:, :, 0, :], psQ[:])
            i_mm = 0
            for ah in range(2):
                a2 = slice(2 * ah, 2 * ah + 2)
                nc.vector.tensor_tensor(
                    out=arg16[:, a2, :, :], in0=preT[:, a2, :, bs],
                    in1=q16[:, a2, :, :].to_broadcast([128, 2, L, GB]),
                    op=OP.add)
                nc.scalar.activation(
                    out=s16[:, a2, :, :], in_=arg16[:, a2, :, :], func=AF.Tanh)
                for ac in range(2 * ah, 2 * ah + 2):
                    for b in range(GB):
                        nc.tensor.matmul(psE[:], lhsT=vshift[:, ac, GB * g + b, bs],
                                         rhs=s16[:, ac, :, b],
                                         start=(i_mm == 0), stop=(i_mm == 4 * GB - 1))
                        i_mm += 1

            # --- softmax over l ---
            usum = work.tile([GB, 1], F32, tag=f"usum{g}")
            nc.scalar.activation(out=scg[g][:, t, :], in_=psE[:], func=AF.Exp,
                                 accum_out=usum[:])
            rcp = work.tile([GB, 1], F32, tag=f"rcp{g}")
            nc.vector.reciprocal(rcp[:], usum[:])
            nc.vector.tensor_scalar_mul(scg[g][:, t, :], scg[g][:, t, :], rcp[:])

            # --- score transpose -> [l, b], wctx ---
            psT = psET[0:L, 25:26, :].rearrange("p a b -> p (a b)")
            nc.tensor.transpose(psT, scg[g][:, t, :], id128[:GB, :GB])
            scT = work.tile([L, GB], F16, tag=f"scT{g}")
            nc.vector.tensor_copy(scT[:], psT)
            for ec in range(4):
                for b in range(GB):
                    nc.tensor.matmul(psW[:, ec, b:b + 1], lhsT=ctxst[:, GB * g + b, ec, :],
                                     rhs=scT[:, b:b + 1], start=True, stop=True)
            nc.vector.tensor_copy(wch16[:, :, t + 1, bs], psW[:])
            if t == t_steps - 1:
                nc.vector.tensor_copy(wlast[:, :, bs], psW[:])

        # -------- Phase E: deferred readout / copy gate / maxout --------
        for c in range(NCH):
            t0 = c * TCH
            pR_t = psP.tile([128, 2, 512], F32, tag="big", name="pR_t")
            pR = pR_t[:, 0, :]
            pC = psP.tile([128, 1], F32, tag="small2")

            def xcat(kc):
                if kc < 4:
                    return embT[:, kc, c * CH:(c + 1) * CH]
                if kc < 8:
                    return hh16[:, t0:t0 + TCH, kc - 4, :]
                return wch16[:, t0 + 1:t0 + TCH + 1, kc - 8, :]

            for kc in range(12):
                nc.tensor.matmul(pR[:CH, :], lhsT=xcat(kc), rhs=wread[:, kc, :],
                                 start=(kc == 0), stop=False)
            nc.tensor.matmul(pR[:CH, :], lhsT=ones1[:, :CH], rhs=bread[:],
                             start=False, stop=True)
            for kc in range(4, 12):
                nc.tensor.matmul(pC[:CH, :], lhsT=xcat(kc), rhs=wcopy[:, kc - 4, :],
                                 start=(kc == 4), stop=False)
            nc.tensor.matmul(pC[:CH, :], lhsT=ones1[:, :CH], rhs=bcopy[:],
                             start=False, stop=True)
            roA = work.tile([CH, 256], F32, tag="roA")
            nc.scalar.copy(roA[:], pR[:CH, 0:256])
            mo = work.tile([CH, 256], F32, tag="mo")
            nc.vector.tensor_tensor(out=mo[:], in0=roA[:], in1=pR[:CH, 256:512], op=OP.max)
            nc.sync.dma_start(out=o_mo[c * CH:(c + 1) * CH, :], in_=mo[:])
            cpt = work.tile([CH, 1], F32, tag="cpt")
            nc.scalar.activation(out=cpt[:], in_=pC[:CH, :], func=AF.Tanh, scale=0.5)
            nc.vector.tensor_scalar(out=cpt[:], in0=cpt[:], scalar1=0.5, scalar2=0.5,
                                    op0=OP.mult, op1=OP.add)
            nc.sync.dma_start(out=o_cp[c * CH:(c + 1) * CH, :], in_=cpt[:])

        # ---------------- final DMAs ----------------
        nc.vector.tensor_copy(hcast[:], hh16[:])
        nc.sync.dma_start(out=o_h[:], in_=hcast[:].rearrange("p k t b -> p (k t b)"))
        nc.sync.dma_start(out=o_sc0[:], in_=scg0[:].rearrange("b t l -> b (t l)"))
        nc.sync.dma_start(out=o_sc1[:], in_=scg1[:].rearrange("b t l -> b (t l)"))
        nc.sync.dma_start(out=o_wl[:], in_=wlast[:].rearrange("p k b -> p (k b)"))

    nc.compile()
    return nc


# ============================ host side ============================

def _prep_core_inputs(inputs, core, t_steps=T):
    b0 = core * BL
    f32 = lambda k: np.asarray(inputs[k], np.float32)
    w_ih, w_hh = f32("W_ih"), f32("W_hh")
    b_ih, b_hh = f32("b_ih"), f32("b_hh")
    w_read, b_read = f32("W_read"), f32("b_read")
    w_copy, b_copy = f32("W_copy"), f32("b_copy")
    w_pre, b_pre = f32("W_pre"), f32("b_pre")
    w_q, w_v = f32("W_q"), f32("W_v")
    ctx = f32("context")[:, b0:b0 + BL, :]                      # [L, BL, E]
    ids = np.asarray(inputs["input_ids"]).astype(np.int32)[:t_steps, b0:b0 + BL]
    h0 = f32("hidden")[0, b0:b0 + BL, :]                        # [BL, D]
    ia = f32("init_att")[b0:b0 + BL, :]

    perm = np.concatenate([np.arange(0, DEC, 2), np.arange(1, DEC, 2)])
    w_read_r, b_read_r = w_read[perm], b_read[perm]

    gibias = np.concatenate([(b_ih + b_hh)[:2 * DEC], b_ih[2 * DEC:]])
    vshift = np.zeros((4, 128, BL, BL), np.float16)
    for c in range(4):
        for b in range(BL):
            vshift[c, :, b, b] = w_v[0, c * 128:(c + 1) * 128].astype(np.float16)

    def chunkT(x):  # [N, D=512] -> [128, 4, N] -> [128, 4*N]
        return np.ascontiguousarray(x.T).reshape(4, 128, -1).transpose(1, 0, 2) \
                 .reshape(128, -1)

    d = {
        "word_lut": f32("word_lut"),
        "ids": ids.reshape(-1, 1),
        "wiheT": np.ascontiguousarray(w_ih[:, :DW].T).astype(np.float16),
        "wihcT": np.ascontiguousarray(w_ih[:, DW:].T).astype(np.float16),
        "whhT": np.ascontiguousarray(w_hh.T).astype(np.float16),
        "wqT": np.ascontiguousarray(w_q.T).astype(np.float16),
        "wpreT": np.ascontiguousarray(w_pre.T).astype(np.float16),
        "wreadT": np.ascontiguousarray(w_read_r.T).astype(np.float16),
        "bread": b_read_r.reshape(1, -1).astype(np.float16),
        "wcopyT": np.ascontiguousarray(w_copy.T).astype(np.float16),
        "bcopy": b_copy.reshape(1, 1).astype(np.float16),
        "gibiasT": np.ascontiguousarray(gibias.reshape(12, 128).T).astype(np.float32),
        "bhhn": b_hh[2 * DEC:].reshape(1, -1).astype(np.float16),
        "bpreT": np.ascontiguousarray(b_pre.reshape(4, 128).T).astype(np.float32),
        "vshift": vshift.transpose(1, 0, 2, 3).reshape(128, -1),
        "ctxstT": ctx.reshape(L, -1).astype(np.float16),
        "ctxmvT": np.ascontiguousarray(ctx.transpose(2, 0, 1)).reshape(4, 128, L, BL)
                    .transpose(1, 0, 2, 3).reshape(128, -1).astype(np.float16),
        "h0T32": chunkT(h0).astype(np.float32),
        "id128": np.eye(128, dtype=np.float32),
        "id128h": np.eye(128, dtype=np.float16),
        "ones1": np.ones((1, 128), np.float16),
    }
    d["h0T16"] = d["h0T32"].astype(np.float16)
    d["iaT16"] = chunkT(ia).astype(np.float16)
    return d


def _assemble(results, t_steps=T):
    g_outputs = np.zeros((t_steps, B, 256), np.float32)
    c_outputs = np.zeros((t_steps, B, L), np.float32)
    copy_gates = np.zeros((t_steps, B, 1), np.float32)
    g_hiddens = np.zeros((t_steps, 1, B, DEC), np.float32)
    ctx_fin = np.zeros((B, ENC), np.float32)
    for c in range(NC):
        r = results[c]
        b0 = c * BL
        g_outputs[:, b0:b0 + BL, :] = r["out_mo"].reshape(t_steps, BL, 256)
        sc = np.concatenate([r["out_score0"].reshape(BL // 2, t_steps, L),
                             r["out_score1"].reshape(BL // 2, t_steps, L)], axis=0)
        c_outputs[:, b0:b0 + BL, :] = sc.transpose(1, 0, 2)
        copy_gates[:, b0:b0 + BL, :] = r["out_cp"].reshape(t_steps, BL, 1)
        hh = r["out_h"].reshape(128, 4, t_steps, BL)
        g_hiddens[:, 0, b0:b0 + BL, :] = hh.transpose(2, 3, 1, 0).reshape(t_steps, BL, DEC)
        wl = r["out_wctx_last"].reshape(128, 4, BL)
        ctx_fin[b0:b0 + BL, :] = wl.transpose(2, 1, 0).reshape(BL, ENC)
    h_fin = g_hiddens[t_steps - 1]
    attn_last = c_outputs[t_steps - 1]
    return (g_outputs, c_outputs, copy_gates, h_fin, attn_last, ctx_fin, g_hiddens)


_NC_CACHE = {}


def kernel(**inputs):
    t_steps = T
    if t_steps not in _NC_CACHE:
        _NC_CACHE[t_steps] = build_nc(t_steps)
    nc = _NC_CACHE[t_steps]
    in_maps = [_prep_core_inputs(inputs, c, t_steps) for c in range(NC)]
    res = run_bass_kernel_spmd(nc, in_maps, core_ids=list(range(NC)))
    return _assemble(res.results, t_steps)


# revision 34
# speedup vs baseline: 1.1183x; 1.0426x over previous
"""Trainium2 Bass kernel for nn_Decoder (GRU + concat-attention decoder).

Strategy: data-parallel over batch across 8 cores (8 examples/core).
Everything SBUF-resident; per-step recurrence fully unrolled.
Feature-on-partition layouts throughout; fp16 matmul operands, fp32
accumulation/elementwise. Readout/copy/maxout deferred to a batched
post-phase. Host does input layout prep (shard/transpose/cast) and
output reassembly; embedding gather runs on device via indirect DMA.
"""
import sys

sys.path.insert(0, "/opt/trn_rl_repo")


from contextlib import ExitStack

import numpy as np

import concourse.bacc as bacc
import concourse.bass as bass
import concourse.tile as tile
from concourse import mybir
from concourse.bass_utils import run_bass_kernel_spmd

F16 = mybir.dt.float16
F32 = mybir.dt.float32
I32 = mybir.dt.int32
AF = mybir.ActivationFunctionType
OP = mybir.AluOpType
AX = mybir.AxisListType

V, DW, ENC, DEC, ATT = 32000, 512, 512, 512, 512
T, B, L = 48, 64, 100
NC = 8
BL = B // NC  # 8 examples per core


def build_nc(t_steps=T):
    nc = bacc.Bacc("TRN2", target_bir_lowering=False)
    tb = t_steps * BL
    CH = min(128, tb)        # (t,b)-row chunk for gather/readout phases
    NCH = tb // CH
    TCH = CH // BL           # timesteps per row chunk

    # ---- DRAM inputs ----
    d_lut = nc.dram_tensor("word_lut", [V, DW], F32, kind="ExternalInput")
    d_ids = nc.dram_tensor("ids", [tb, 1], I32, kind="ExternalInput")
    d_wihe = nc.dram_tensor("wiheT", [DW, 3 * DEC], F16, kind="ExternalInput")
    d_wihc = nc.dram_tensor("wihcT", [ENC, 3 * DEC], F16, kind="ExternalInput")
    d_whh = nc.dram_tensor("whhT", [DEC, 3 * DEC], F16, kind="ExternalInput")
    d_wq = nc.dram_tensor("wqT", [DEC, ATT], F16, kind="ExternalInput")
    d_wpre = nc.dram_tensor("wpreT", [ENC, ATT], F16, kind="ExternalInput")
    d_wread = nc.dram_tensor("wreadT", [DW + DEC + ENC, DEC], F16, kind="ExternalInput")
    d_bread = nc.dram_tensor("bread", [1, DEC], F16, kind="ExternalInput")
    d_wcopy = nc.dram_tensor("wcopyT", [DEC + ENC, 1], F16, kind="ExternalInput")
    d_bcopy = nc.dram_tensor("bcopy", [1, 1], F16, kind="ExternalInput")
    d_gibias = nc.dram_tensor("gibiasT", [128, 12], F32, kind="ExternalInput")
    d_bhhn = nc.dram_tensor("bhhn", [1, DEC], F16, kind="ExternalInput")
    d_bpre = nc.dram_tensor("bpreT", [128, 4], F32, kind="ExternalInput")
    d_vshift = nc.dram_tensor("vshift", [128, 4 * BL * BL], F16, kind="ExternalInput")
    d_ctxst = nc.dram_tensor("ctxstT", [L, BL * ENC], F16, kind="ExternalInput")
    d_ctxmv = nc.dram_tensor("ctxmvT", [128, 4 * BL * L], F16, kind="ExternalInput")
    d_h0_32 = nc.dram_tensor("h0T32", [128, 4 * BL], F32, kind="ExternalInput")
    d_h0_16 = nc.dram_tensor("h0T16", [128, 4 * BL], F16, kind="ExternalInput")
    d_ia_16 = nc.dram_tensor("iaT16", [128, 4 * BL], F16, kind="ExternalInput")
    d_id128 = nc.dram_tensor("id128", [128, 128], F32, kind="ExternalInput")
    d_id128h = nc.dram_tensor("id128h", [128, 128], F16, kind="ExternalInput")
    d_ones = nc.dram_tensor("ones1", [1, 128], F16, kind="ExternalInput")

    # ---- DRAM outputs ----
    o_mo = nc.dram_tensor("out_mo", [tb, 256], F32, kind="ExternalOutput")
    o_sc0 = nc.dram_tensor("out_score0", [BL // 2, t_steps * L], F32, kind="ExternalOutput")
    o_sc1 = nc.dram_tensor("out_score1", [BL // 2, t_steps * L], F32, kind="ExternalOutput")
    o_h = nc.dram_tensor("out_h", [128, t_steps * 4 * BL], F32, kind="ExternalOutput")
    o_wl = nc.dram_tensor("out_wctx_last", [128, 4 * BL], F32, kind="ExternalOutput")
    o_cp = nc.dram_tensor("out_cp", [tb, 1], F32, kind="ExternalOutput")

    with tile.TileContext(nc) as tc, ExitStack() as ctx:
        konst = ctx.enter_context(tc.tile_pool(name="konst", bufs=1))
        work = ctx.enter_context(tc.tile_pool(name="work", bufs=2))
        psL = ctx.enter_context(tc.tile_pool(name="psL", bufs=1, space="PSUM"))
        psD = ctx.enter_context(tc.tile_pool(name="psD", bufs=2, space="PSUM"))
        psP = psL

        def load(name, dram, shape, dtype, in_ap=None):
            t_ = konst.tile(shape, dtype, tag=name)
            nc.sync.dma_start(out=t_[:], in_=dram[:] if in_ap is None else in_ap)
            return t_

        wihe = load("wihe", d_wihe, [128, 4, 1536], F16,
                    d_wihe.rearrange("(k p) g -> p k g", p=128))
        wihc = load("wihc", d_wihc, [128, 4, 1536], F16,
                    d_wihc.rearrange("(k p) g -> p k g", p=128))
        whh = load("whh", d_whh, [128, 4, 1536], F16,
                   d_whh.rearrange("(k p) g -> p k g", p=128))
        wq = load("wq", d_wq, [128, 4, 512], F16,
                  d_wq.rearrange("(k p) g -> p k g", p=128))
        wpre = load("wpre", d_wpre, [128, 4, 512], F16,
                    d_wpre.rearrange("(k p) g -> p k g", p=128))
        wread = load("wread", d_wread, [128, 12, 512], F16,
                     d_wread.rearrange("(k p) g -> p k g", p=128))
        wcopy = load("wcopy", d_wcopy, [128, 8, 1], F16,
                     d_wcopy.rearrange("(k p) g -> p k g", p=128))
        bread = load("bread", d_bread, [1, 512], F16)
        bcopy = load("bcopy", d_bcopy, [1, 1], F16)
        gibias = load("gibias", d_gibias, [128, 12], F32)
        bhhn = load("bhhn", d_bhhn, [1, 4, 128], F16,
                    d_bhhn.rearrange("o (k p) -> o k p", p=128))
        bpre = load("bpre", d_bpre, [128, 4], F32)
        vshift = load("vshift", d_vshift, [128, 4, BL, BL], F16,
                      d_vshift.rearrange("p (k i j) -> p k i j", k=4, i=BL))
        ctxst = load("ctxst", d_ctxst, [L, BL, 4, 128], F16,
                     d_ctxst.rearrange("l (b k p) -> l b k p", b=BL, k=4))
        ctxmv = load("ctxmv", d_ctxmv, [128, 4, L, BL], F16,
                     d_ctxmv.rearrange("p (k l b) -> p k l b", k=4, l=L))
        h0_32 = load("h0_32", d_h0_32, [128, 4, BL], F32,
                     d_h0_32.rearrange("p (k b) -> p k b", k=4))
        h0_16 = load("h0_16", d_h0_16, [128, 4, BL], F16,
                     d_h0_16.rearrange("p (k b) -> p k b", k=4))
        ia16 = load("ia16", d_ia_16, [128, 4, BL], F16,
                    d_ia_16.rearrange("p (k b) -> p k b", k=4))
        id128 = load("id128", d_id128, [128, 128], F32)
        id128h = load("id128h", d_id128h, [128, 128], F16)
        ones1 = load("ones1", d_ones, [1, 128], F16)
        ids_sb = load("ids_sb", d_ids, [CH, NCH, 1], I32,
                      d_ids.rearrange("(c p) o -> p c o", p=CH))

        # ---- persistent state / history tiles ----
        embT = konst.tile([128, 4, tb], F16, tag="embT")
        giemb = konst.tile([128, 12, t_steps, BL], F16, tag="giemb")
        preT = konst.tile([128, 4, L, BL], F16, tag="preT")
        hcast = konst.tile([128, 4, t_steps, BL], F32, tag="hcast")
        hh16 = konst.tile([128, t_steps, 4, BL], F16, tag="hh16")
        wch16 = konst.tile([128, t_steps + 1, 4, BL], F16, tag="wch16")
        scg0 = konst.tile([BL // 2, t_steps, L], F32, tag="scg0")
        scg1 = konst.tile([BL // 2, t_steps, L], F32, tag="scg1")
        scg = [scg0, scg1]
        wlast = konst.tile([128, 4, BL], F32, tag="wlast")
        embrows = konst.tile([CH, NCH, 512], F32, tag="embrows")

        # ---------------- Phase A: embedding gather + transpose ----------------
        for c in range(NCH):
            nc.gpsimd.indirect_dma_start(
                out=embrows[:, c, :],
                out_offset=None,
                in_=d_lut[:],
                in_offset=bass.IndirectOffsetOnAxis(ap=ids_sb[:, c, :], axis=0),
            )
        for c in range(NCH):
            for kc in range(4):
                pt = psL.tile([128, CH], F32, tag="small1")
                nc.tensor.transpose(pt[:], embrows[:, c, kc * 128:(kc + 1) * 128],
                                    id128[:CH, :CH])
                nc.vector.tensor_copy(embT[:, kc, c * CH:(c + 1) * CH], pt[:])

        # ---------------- Phase B: gi_emb = W_ihe @ emb + biases ----------------
        for h4 in range(6):
            pg = psP.tile([128, 2, 512], F32, tag="big")
            for g3 in range(2):
                gc = 2 * h4 + g3
                for kc in range(4):
                    nc.tensor.matmul(
                        pg[:, g3, :tb], lhsT=wihe[:, kc, gc * 128:(gc + 1) * 128],
                        rhs=embT[:, kc, :], start=(kc == 0), stop=(kc == 3))
            nc.vector.tensor_tensor(
                out=giemb[:, 2 * h4:2 * h4 + 2, :, :].rearrange("p a b c -> p a (b c)"),
                in0=pg[:, :, :tb],
                in1=gibias[:, 2 * h4:2 * h4 + 2].to_broadcast([128, 2, tb]),
                op=OP.add)

        # ---------------- Phase C: pre = W_pre @ ctx + b_pre ----------------
        for ac in range(4):
            for lh in range(2):
                pp_t = psP.tile([128, 2, 512], F32, tag="big", name="pp_t")
                pp = pp_t[:, 0, :]
                for kc in range(4):
                    nc.tensor.matmul(
                        pp[:, :50 * BL],
                        lhsT=wpre[:, kc, ac * 128:(ac + 1) * 128],
                        rhs=ctxmv[:, kc, 50 * lh:50 * lh + 50, :].rearrange("p l b -> p (l b)"),
                        start=(kc == 0), stop=(kc == 3))
                nc.scalar.activation(
                    out=preT[:, ac, 50 * lh:50 * lh + 50, :].rearrange("p l b -> p (l b)"),
                    in_=pp[:, :50 * BL], func=AF.Identity, bias=bpre[:, ac:ac + 1])

        # ---------------- Phase D: recurrence ----------------
        # Two independent batch sub-groups of 4 pipeline against each other:
        # group A's DVE/ACT attention overlaps group B's PE gate matmuls.
        GB = BL // 2
        for t in range(t_steps):
          for g in range(2):
            bs = slice(GB * g, GB * (g + 1))
            hp16 = (lambda kc: h0_16[:, kc, bs]) if t == 0 else \
                (lambda kc, _t=t: hh16[:, kc, _t - 1, bs])
            hprev16 = h0_16[:, :, bs] if t == 0 else hh16[:, :, t - 1, bs]
            wp16 = (lambda kc: ia16[:, kc, bs]) if t == 0 else \
                (lambda kc, _t=t: wch16[:, kc, _t, bs])

            # one PSUM bank per group: [gates 16 | q 4 | wctx 4] x GB
            psZ = psD.tile([128, 76, GB], F32, tag=f"psZ{g}")
            psG = psZ[:, 0:16, :]
            psA = psZ[:, 0:8, :]
            psB = psZ[:, 8:12, :]
            psC = psZ[:, 12:16, :]
            psQ = psZ[:, 16:20, :]
            psW = psZ[:, 20:24, :]
            psE = psZ[0:GB, 24:49, :].rearrange("p a b -> p (a b)")

            for gc in range(8):
                for kc in range(4):
                    nc.tensor.matmul(psA[:, gc, :], lhsT=whh[:, kc, gc * 128:(gc + 1) * 128],
                                     rhs=hp16(kc), start=(kc == 0), stop=False)
                for kc in range(4):
                    nc.tensor.matmul(psA[:, gc, :], lhsT=wihc[:, kc, gc * 128:(gc + 1) * 128],
                                     rhs=wp16(kc), start=False, stop=False)
                nc.tensor.matmul(psA[:, gc, :], lhsT=id128h[:], rhs=giemb[:, gc, t, bs],
                                 start=False, stop=True)
            for gn in range(4):
                gc = 8 + gn
                for kc in range(4):
                    nc.tensor.matmul(psB[:, gn, :], lhsT=wihc[:, kc, gc * 128:(gc + 1) * 128],
                                     rhs=wp16(kc), start=(kc == 0), stop=False)
                nc.tensor.matmul(psB[:, gn, :], lhsT=id128h[:], rhs=giemb[:, 8 + gn, t, bs],
                                 start=False, stop=True)
                for kc in range(4):
                    nc.tensor.matmul(psC[:, gn, :], lhsT=whh[:, kc, gc * 128:(gc + 1) * 128],
                                     rhs=hp16(kc), start=(kc == 0), stop=False)
                nc.tensor.matmul(psC[:, gn, :], lhsT=bhhn[:, gn, :], rhs=ones1[:, :GB],
                                 start=False, stop=True)

            # --- GRU elementwise ---
            rz = work.tile([128, 8, GB], F32, tag=f"rz{g}")
            nc.scalar.activation(out=rz[:], in_=psA[:], func=AF.Tanh, scale=0.5)
            t1 = work.tile([128, 4, GB], F32, tag=f"t1{g}")
            nc.vector.scalar_tensor_tensor(out=t1[:], in0=rz[:, 0:4, :], scalar=1.0,
                                           in1=psC[:], op0=OP.add, op1=OP.mult)
            narg = work.tile([128, 4, GB], F32, tag=f"narg{g}")
            nc.vector.tensor_tensor(out=narg[:], in0=t1[:], in1=psB[:], op=OP.add)
            nn_ = work.tile([128, 4, GB], F32, tag=f"nn{g}")
            nc.scalar.activation(out=nn_[:], in_=narg[:], func=AF.Tanh)
            dd = work.tile([128, 4, GB], F32, tag=f"dd{g}")
            nc.vector.tensor_tensor(out=dd[:], in0=hprev16, in1=nn_[:], op=OP.subtract)
            nc.vector.scalar_tensor_tensor(out=dd[:], in0=rz[:, 4:8, :], scalar=1.0,
                                           in1=dd[:], op0=OP.add, op1=OP.mult)
            nc.vector.scalar_tensor_tensor(out=hh16[:, :, t, bs], in0=dd[:], scalar=0.5,
                                           in1=nn_[:], op0=OP.mult, op1=OP.add)

            # --- q = h1 @ W_q.T ---
            for ac in range(4):
                for kc in range(4):
                    nc.tensor.matmul(psQ[:, ac, :], lhsT=wq[:, kc, ac * 128:(ac + 1) * 128],
                                     rhs=hh16[:, kc, t, bs], start=(kc == 0), stop=(kc == 3))

            # --- arg = pre + q (broadcast over l), tanh, energy ---
            arg16 = work.tile([128, 4, L, GB], F16, tag=f"arg16{g}")
            s16 = work.tile([128, 4, L, GB], F16, tag=f"s16{g}")
            q16 = work.tile([128, 4, 1, GB], F16, tag=f"q16{g}")
            nc.vector.tensor_copy(q16[:, :, 0, :], psQ[:])
            i_mm = 0
            for ah in range(2):
                a2 = slice(2 * ah, 2 * ah + 2)
                nc.vector.tensor_tensor(
                    out=arg16[:, a2, :, :], in0=preT[:, a2, :, bs],
                    in1=q16[:, a2, :, :].to_broadcast([128, 2, L, GB]),
                    op=OP.add)
                nc.scalar.activation(
                    out=s16[:, a2, :, :], in_=arg16[:, a2, :, :], func=AF.Tanh)
                for ac in range(2 * ah, 2 * ah + 2):
                    for b in range(GB):
                        nc.tensor.matmul(psE[:], lhsT=vshift[:, ac, GB * g + b, bs],
                                         rhs=s16[:, ac, :, b],
                                         start=(i_mm == 0), stop=(i_mm == 4 * GB - 1))
                        i_mm += 1

            # --- softmax over l ---
            usum = work.tile([GB, 1], F32, tag=f"usum{g}")
            nc.scalar.activation(out=scg[g][:, t, :], in_=psE[:], func=AF.Exp,
                                 accum_out=usum[:])
            rcp = work.tile([GB, 1], F32, tag=f"rcp{g}")
            nc.vector.reciprocal(rcp[:], usum[:])
            nc.vector.tensor_scalar_mul(scg[g][:, t, :], scg[g][:, t, :], rcp[:])

            # --- score transpose -> [l, b], wctx ---
            psT = psZ[0:L, 49:50, :].rearrange("p a b -> p (a b)")
            nc.tensor.transpose(psT, scg[g][:, t, :], id128[:GB, :GB])
            scT = work.tile([L, GB], F16, tag=f"scT{g}")
            nc.vector.tensor_copy(scT[:], psT)
            for ec in range(4):
                for b in range(GB):
                    nc.tensor.matmul(psW[:, ec, b:b + 1], lhsT=ctxst[:, GB * g + b, ec, :],
                                     rhs=scT[:, b:b + 1], start=True, stop=True)
            nc.vector.tensor_copy(wch16[:, :, t + 1, bs], psW[:])
            if t == t_steps - 1:
                nc.vector.tensor_copy(wlast[:, :, bs], psW[:])

        # -------- Phase E: deferred readout / copy gate / maxout --------
        for c in range(NCH):
            t0 = c * TCH
            pR_t = psP.tile([128, 2, 512], F32, tag="big", name="pR_t")
            pR = pR_t[:, 0, :]
            pC = psP.tile([128, 1], F32, tag="small2")

            def xcat(kc):
                if kc < 4:
                    return embT[:, kc, c * CH:(c + 1) * CH]
                if kc < 8:
                    return hh16[:, t0:t0 + TCH, kc - 4, :]
                return wch16[:, t0 + 1:t0 + TCH + 1, kc - 8, :]

            for kc in range(12):
                nc.tensor.matmul(pR[:CH, :], lhsT=xcat(kc), rhs=wread[:, kc, :],
                                 start=(kc == 0), stop=False)
            nc.tensor.matmul(pR[:CH, :], lhsT=ones1[:, :CH], rhs=bread[:],
                             start=False, stop=True)
            for kc in range(4, 12):
                nc.tensor.matmul(pC[:CH, :], lhsT=xcat(kc), rhs=wcopy[:, kc - 4, :],
                                 start=(kc == 4), stop=False)
            nc.tensor.matmul(pC[:CH, :], lhsT=ones1[:, :CH], rhs=bcopy[:],
                             start=False, stop=True)
            roA = work.tile([CH, 256], F32, tag="roA")
            nc.scalar.copy(roA[:], pR[:CH, 0:256])
            mo = work.tile([CH, 256], F32, tag="mo")
            nc.vector.tensor_tensor(out=mo[:], in0=roA[:], in1=pR[:CH, 256:512], op=OP.max)
            nc.sync.dma_start(out=o_mo[c * CH:(c + 1) * CH, :], in_=mo[:])
            cpt = work.tile([CH, 1], F32, tag="cpt")
            nc.scalar.activation(out=cpt[:], in_=pC[:CH, :], func=AF.Tanh, scale=0.5)
            nc.vector.tensor_scalar(out=cpt[:], in0=cpt[:], scalar1=0.5, scalar2=0.5,
                                    op0=OP.mult, op1=OP.add)
            nc.sync.dma_start(out=o_cp[c * CH:(c + 1) * CH, :], in_=cpt[:])

        # ---------------- final DMAs ----------------
        nc.vector.tensor_copy(hcast[:], hh16[:])
        nc.sync.dma_start(out=o_h[:], in_=hcast[:].rearrange("p k t b -> p (k t b)"))
        nc.sync.dma_start(out=o_sc0[:], in_=scg0[:].rearrange("b t l -> b (t l)"))
        nc.sync.dma_start(out=o_sc1[:], in_=scg1[:].rearrange("b t l -> b (t l)"))
        nc.sync.dma_start(out=o_wl[:], in_=wlast[:].rearrange("p k b -> p (k b)"))

    nc.compile()
    return nc


# ============================ host side ============================

def _prep_core_inputs(inputs, core, t_steps=T):
    b0 = core * BL
    f32 = lambda k: np.asarray(inputs[k], np.float32)
    w_ih, w_hh = f32("W_ih"), f32("W_hh")
    b_ih, b_hh = f32("b_ih"), f32("b_hh")
    w_read, b_read = f32("W_read"), f32("b_read")
    w_copy, b_copy = f32("W_copy"), f32("b_copy")
    w_pre, b_pre = f32("W_pre"), f32("b_pre")
    w_q, w_v = f32("W_q"), f32("W_v")
    ctx = f32("context")[:, b0:b0 + BL, :]                      # [L, BL, E]
    ids = np.asarray(inputs["input_ids"]).astype(np.int32)[:t_steps, b0:b0 + BL]
    h0 = f32("hidden")[0, b0:b0 + BL, :]                        # [BL, D]
    ia = f32("init_att")[b0:b0 + BL, :]

    perm = np.concatenate([np.arange(0, DEC, 2), np.arange(1, DEC, 2)])
    w_read_r, b_read_r = w_read[perm], b_read[perm]

    gibias = np.concatenate([(b_ih + b_hh)[:2 * DEC], b_ih[2 * DEC:]])
    vshift = np.zeros((4, 128, BL, BL), np.float16)
    for c in range(4):
        for b in range(BL):
            vshift[c, :, b, b] = w_v[0, c * 128:(c + 1) * 128].astype(np.float16)

    def chunkT(x):  # [N, D=512] -> [128, 4, N] -> [128, 4*N]
        return np.ascontiguousarray(x.T).reshape(4, 128, -1).transpose(1, 0, 2) \
                 .reshape(128, -1)

    d = {
        "word_lut": f32("word_lut"),
        "ids": ids.reshape(-1, 1),
        "wiheT": np.ascontiguousarray(w_ih[:, :DW].T).astype(np.float16),
        "wihcT": np.ascontiguousarray(w_ih[:, DW:].T).astype(np.float16),
        "whhT": np.ascontiguousarray(
            (w_hh.T * np.concatenate([np.ones(2 * DEC), np.full(DEC, 0.5)])[None, :])
        ).astype(np.float16),
        "wqT": np.ascontiguousarray(w_q.T).astype(np.float16),
        "wpreT": np.ascontiguousarray(w_pre.T).astype(np.float16),
        "wreadT": np.ascontiguousarray(w_read_r.T).astype(np.float16),
        "bread": b_read_r.reshape(1, -1).astype(np.float16),
        "wcopyT": np.ascontiguousarray(w_copy.T).astype(np.float16),
        "bcopy": b_copy.reshape(1, 1).astype(np.float16),
        "gibiasT": np.ascontiguousarray(gibias.reshape(12, 128).T).astype(np.float32),
        "bhhn": (0.5 * b_hh[2 * DEC:]).reshape(1, -1).astype(np.float16),
        "bpreT": np.ascontiguousarray(b_pre.reshape(4, 128).T).astype(np.float32),
        "vshift": vshift.transpose(1, 0, 2, 3).reshape(128, -1),
        "ctxstT": ctx.reshape(L, -1).astype(np.float16),
        "ctxmvT": np.ascontiguousarray(ctx.transpose(2, 0, 1)).reshape(4, 128, L, BL)
                    .transpose(1, 0, 2, 3).reshape(128, -1).astype(np.float16),
        "h0T32": chunkT(h0).astype(np.float32),
        "id128": np.eye(128, dtype=np.float32),
        "id128h": np.eye(128, dtype=np.float16),
        "ones1": np.ones((1, 128), np.float16),
    }
    d["h0T16"] = d["h0T32"].astype(np.float16)
    d["iaT16"] = chunkT(ia).astype(np.float16)
    return d


def _assemble(results, t_steps=T):
    g_outputs = np.zeros((t_steps, B, 256), np.float32)
    c_outputs = np.zeros((t_steps, B, L), np.float32)
    copy_gates = np.zeros((t_steps, B, 1), np.float32)
    g_hiddens = np.zeros((t_steps, 1, B, DEC), np.float32)
    ctx_fin = np.zeros((B, ENC), np.float32)
    for c in range(NC):
        r = results[c]
        b0 = c * BL
        g_outputs[:, b0:b0 + BL, :] = r["out_mo"].reshape(t_steps, BL, 256)
        sc = np.concatenate([r["out_score0"].reshape(BL // 2, t_steps, L),
                             r["out_score1"].reshape(BL // 2, t_steps, L)], axis=0)
        c_outputs[:, b0:b0 + BL, :] = sc.transpose(1, 0, 2)
        copy_gates[:, b0:b0 + BL, :] = r["out_cp"].reshape(t_steps, BL, 1)
        hh = r["out_h"].reshape(128, 4, t_steps, BL)
        g_hiddens[:, 0, b0:b0 + BL, :] = hh.transpose(2, 3, 1, 0).reshape(t_steps, BL, DEC)
        wl = r["out_wctx_last"].reshape(128, 4, BL)
        ctx_fin[b0:b0 + BL, :] = wl.transpose(2, 1, 0).reshape(BL, ENC)
    h_fin = g_hiddens[t_steps - 1]
    attn_last = c_outputs[t_steps - 1]
    return (g_outputs, c_outputs, copy_gates, h_fin, attn_last, ctx_fin, g_hiddens)


_NC_CACHE = {}


def kernel(**inputs):
    t_steps = T
    if t_steps not in _NC_CACHE:
        _NC_CACHE[t_steps] = build_nc(t_steps)
    nc = _NC_CACHE[t_steps]
    in_maps = [_prep_core_inputs(inputs, c, t_steps) for c in range(NC)]
    res = run_bass_kernel_spmd(nc, in_maps, core_ids=list(range(NC)))
    return _assemble(res.results, t_steps)
